# revision 16
# baseline (speedup 1.0000x reference)
"""Trainium2 Bass kernel for nn_CPI_CLS_49478023250092 (gnn_message_passing).

Strategy (8 cores, SPMD):
  - GNN: adjacency row-sharded; each core holds A_c.T (4096x512, bf16,
    pre-chunked on host into [128, 32*512]) resident in SBUF, computes
    delta.T = (A_c @ hs).T = sum_k hs_chunk.T @ A_cT_chunk on the tensor
    engine in bf16; per-layer AllGather of the bf16 [10,512] delta
    recovers the full xs.T on every core.  3 layers.
  - Protein conv: L-sharded with 33-col halos (zero at global edges).
    23x23 conv over a [L,10] image = TWO accumulating bf16 matmuls per
    512-col tile against a 12-shift stacked image X12 [120, L].
  - Attention: two-pass tail.  Pass 1 (overlaps the compound AllReduce):
    hsp = relu(W_att @ conv_out + b) for all 16 tiles.  Pass 2: 16
    weight matmuls into one [16,512] PSUM bank, one batched tanh, then
    per tile a broadcast matmul + fused multiply-reduce
    (tensor_tensor_reduce) accumulating the weighted mean.
  - Fusion MLP in f32 on every core; tiny AllReduces for compound and
    protein partial means.
  - Host side does only data movement: embedding gathers, sharding,
    transposition, Toeplitz construction, dtype casts.
"""

import sys
import os

for _p in ("/opt/trn_rl_repo",):
    if _p not in sys.path and os.path.isdir(_p):
        sys.path.insert(0, _p)

import numpy as np
import ml_dtypes

import concourse.bacc as bacc
import concourse.mybir as mybir
from concourse import tile
from concourse.bass_utils import run_bass_kernel_spmd

BF16 = ml_dtypes.bfloat16

NCORES = 8
NA = 4096          # atoms
D = 10             # embed dim
L = 65536          # words
KK = 23            # conv kernel
PAD = 11
R = NA // NCORES   # 512 adjacency rows per core
NCH = NA // 128    # 32 k-chunks
LC = L // NCORES   # 8192 conv columns per core
HALO = 33
LBUF = LC + 2 * HALO   # 8258
T = 512            # free-dim tile
NT = LC // T       # 16 attention tiles

F32 = mybir.dt.float32
BF = mybir.dt.bfloat16

# ---- smalls layout (f32 [128, 100]) ----
# cols 0-9   : watT f32 [10,10]
# col  10    : batt [10,1]
# cols 11-30 : woa0 [10,20] = W_out0[:, :10].T
# cols 31-50 : wob0 [10,20] = W_out0[:, 10:].T
# col  51    : bo0 [20,1]
# cols 52-71 : woT1 [20,20]
# cols 72-91 : woT2 [20,20]
# col 92     : bo1 ; col 93 : bo2
# cols 94-95 : wiT [20,2]
# col  96    : bi [2,1]
# cols 97-99 : conv biases [10,1] for layers 1..3
SM_COLS = 100
# ---- gm layout (bf16 [120, 110]) ----
# cols 20l+0..9  : G0_l [120,10] ; cols 20l+10..19 : G1_l [110,10] (padded)
# cols 60-69     : watT bf16 [10,10]
# cols 70+10l    : wgT_bf[l] [11,10] (W_gnn_w[l].T stacked with bias row)
# cols 100-109   : ones_sc [1,10] at partition 0 (value 1/65536)
GM_COLS = 110

_BUILD_CACHE = {}


def _conv_spans():
    """Per conv layer (1..3): (in_lo, in_hi, out_lo, out_hi) in buffer coords."""
    spans = []
    for l in (1, 2, 3):
        in_lo = 11 * (l - 1)
        in_hi = LBUF - 11 * (l - 1)
        out_lo = 11 * l
        out_hi = LBUF - 11 * l
        spans.append((in_lo, in_hi, out_lo, out_hi))
    return spans


def _tiles(lo, hi, step):
    out = []
    c = lo
    while c < hi:
        out.append((c, min(step, hi - c)))
        c += step
    return out


def build_program():
    # tensor_tensor_reduce hangs real HW (works in CoreSim) — keep it off
    TTR = os.environ.get("K_TTR", "0") == "1"
    VRELU = os.environ.get("K_VRELU", "1") == "1"  # DVE relu variants
    key = ("nc", TTR, VRELU)
    if key in _BUILD_CACHE:
        return _BUILD_CACHE[key]

    nc = bacc.Bacc("TRN2", target_bir_lowering=False, debug=False,
                   num_devices=NCORES)

    xsT0 = nc.dram_tensor("xsT0", [11, NA], BF, kind="ExternalInput").ap()
    a_p = nc.dram_tensor("a_p", [128, NCH * T], BF, kind="ExternalInput").ap()
    wsT = nc.dram_tensor("wsT", [D, LBUF], BF, kind="ExternalInput").ap()
    x12h = nc.dram_tensor("x12h", [120, LBUF], BF, kind="ExternalInput").ap()
    gm = nc.dram_tensor("gm", [120, GM_COLS], BF, kind="ExternalInput").ap()
    smalls = nc.dram_tensor("smalls", [128, SM_COLS], F32,
                            kind="ExternalInput").ap()
    out_d = nc.dram_tensor("out", [1, 2], F32, kind="ExternalOutput").ap()

    spans = _conv_spans()
    rg = [list(range(NCORES))]
    AF = mybir.ActivationFunctionType
    ALU = mybir.AluOpType

    with tile.TileContext(nc) as tc:
        with (
            tc.tile_pool(name="const", bufs=1) as constp,
            tc.tile_pool(name="abuf", bufs=1) as abufp,
            tc.tile_pool(name="ximg", bufs=1) as ximgp,
            tc.tile_pool(name="x12", bufs=1) as x12p,
            tc.tile_pool(name="hs", bufs=1) as hsp_pool,
            tc.tile_pool(name="dl", bufs=2) as dlp,
            tc.tile_pool(name="att", bufs=2) as attp,
            tc.tile_pool(name="misc", bufs=2) as miscp,
            tc.tile_pool(name="ps_mix", bufs=4, space="PSUM") as ps_mix,
            tc.tile_pool(name="ps_dl", bufs=1, space="PSUM") as ps_dl,
            tc.tile_pool(name="ps_sm", bufs=1, space="PSUM") as ps_sm,
            tc.tile_pool(name="ps_wr", bufs=2, space="PSUM") as ps_wr,
            tc.tile_pool(name="dram", bufs=1, space="DRAM") as dram,
        ):
            # ---------------- warmup collective (first!) ----------------
            # the first collective pays a large fixed init + skew barrier;
            # trigger it before anything else so it overlaps the loads
            warm = miscp.tile([1, 8], F32, tag="warm")
            nc.vector.memset(warm[:], 0.0)
            wr_in = dram.tile([1, 8], F32, tag="wrin")
            wr_out = dram.tile([1, 8], F32, tag="wrout")
            nc.scalar.dma_start(wr_in[:], warm[:])
            nc.gpsimd.collective_compute(
                "AllReduce", ALU.add,
                ins=[wr_in.opt()], outs=[wr_out.opt()],
                replica_groups=rg)

            # ---------------- load phase ----------------
            sm = constp.tile([128, SM_COLS], F32, tag="sm")
            nc.sync.dma_start(sm[:], smalls[:])
            gmt = constp.tile([120, GM_COLS], BF, tag="gm")
            nc.sync.dma_start(gmt[:], gm[:])
            xsT = constp.tile([11, NA], BF, tag="xsT")
            nc.sync.dma_start(xsT[:], xsT0[:])
            ximg = ximgp.tile([D, LBUF], BF, tag="ximg")
            nc.sync.dma_start(ximg[:], wsT[:])

            a_sb = abufp.tile([128, NCH * T], BF, tag="a")
            ADMA = 4  # chunks per DMA
            for i, c in enumerate(range(0, NCH, ADMA)):
                eng = nc.sync if i % 2 == 0 else nc.scalar
                eng.dma_start(a_sb[:, c * T:(c + ADMA) * T],
                              a_p[:, c * T:(c + ADMA) * T])

            x12 = x12p.tile([120, LBUF], BF, tag="x12")

            # collective bounce buffers
            cc_in = [dram.tile([D, T], BF, tag=f"ccin{i}",
                               name=f"ccin{i}") for i in range(2)]
            cc_out = [dram.tile([8 * D, T], BF, tag=f"ccout{i}",
                                name=f"ccout{i}") for i in range(2)]
            ar_c_in = dram.tile([D, 8], F32, tag="arcin")
            ar_c_out = dram.tile([D, 8], F32, tag="arcout")
            ar_p_in = dram.tile([D, 8], F32, tag="arpin")
            ar_p_out = dram.tile([D, 8], F32, tag="arpout")

            wgT = [gmt[0:11, 70 + 10 * l:80 + 10 * l] for l in range(3)]
            watT = sm[0:D, 0:10]
            batt = sm[0:D, 10:11]
            watT_bf = gmt[0:D, 60:70]
            ones_bf = gmt[0:1, 100:110]
            cbias = [sm[0:D, 97 + i:98 + i] for i in range(3)]

            def build_x12(l):
                if l == 1:
                    # layer-1 stack comes pre-built from the host: one big
                    # DMA with 128 descriptors spreads across all engines
                    nc.sync.dma_start(x12[:], x12h[:])
                    return
                # on-device rebuild: gpsimd/SWDGE spreads the 12 shifted
                # copies across all DMA rings (~8us vs ~29us on HWDGE)
                in_lo, in_hi, _, _ = spans[l - 1]
                for p in range(12):
                    nc.gpsimd.dma_start(
                        x12[10 * p:10 * p + 10, in_lo:in_hi - p],
                        ximg[:, in_lo + p:in_hi])

            _cv_count = [0]

            def conv_layer(l):
                in_lo, in_hi, out_lo, out_hi = spans[l - 1]
                g0 = gmt[0:120, 20 * (l - 1):20 * (l - 1) + 10]
                g1 = gmt[0:110, 20 * (l - 1) + 10:20 * (l - 1) + 20]
                for (b0, tw) in _tiles(out_lo, out_hi, T):
                    ps = ps_mix.tile([D, T], F32, tag="mix")
                    nc.tensor.matmul(ps[:, :tw], g0,
                                     x12[0:120, b0 - 11:b0 - 11 + tw],
                                     start=True, stop=False)
                    nc.tensor.matmul(ps[:, :tw], g1,
                                     x12[0:110, b0 + 1:b0 + 1 + tw],
                                     start=False, stop=True)
                    _cv_count[0] += 1
                    if VRELU and _cv_count[0] % 2 == 1:
                        nc.vector.tensor_scalar(ximg[:, b0:b0 + tw],
                                                ps[:, :tw], cbias[l - 1], 0.0,
                                                op0=ALU.add, op1=ALU.max)
                    else:
                        nc.scalar.activation(ximg[:, b0:b0 + tw], ps[:, :tw],
                                             AF.Relu, bias=cbias[l - 1])

            def gnn_layer(l):
                """Interleaved hs+delta matmuls; returns delta psum."""
                hs_sb = hsp_pool.tile([128, NCH * D], BF, tag="hs")
                dl_ps = ps_dl.tile([D, T], F32, tag="dl")
                for c in range(NCH):
                    hp = ps_mix.tile([128, D], F32, tag="mix", name="hp")
                    nc.tensor.matmul(hp[:], xsT[:, 128 * c:128 * (c + 1)],
                                     wgT[l])
                    dst = hs_sb[:, D * c:D * (c + 1)]
                    if VRELU:
                        nc.vector.tensor_scalar_max(dst, hp[:], 0.0)
                    else:
                        nc.scalar.activation(dst, hp[:], AF.Relu)
                    nc.tensor.matmul(dl_ps[:], dst,
                                     a_sb[:, T * c:T * (c + 1)],
                                     start=(c == 0), stop=(c == NCH - 1))
                return dl_ps

            def stage_delta(dl_ps, idx):
                dcp = dlp.tile([D, T], BF, tag="dcp")
                nc.vector.tensor_scalar_add(dcp[:], dl_ps[:], 0.0)
                nc.gpsimd.dma_start(cc_in[idx][:], dcp[:])

            def apply_delta(idx):
                """DMA gathered deltas back and add into xsT."""
                dT = dlp.tile([D, NA], BF, tag="dT")
                nc.sync.dma_start(
                    dT[:].rearrange("j (r n) -> j r n", r=NCORES),
                    cc_out[idx][:].rearrange("(r j) n -> j r n", j=D))
                nc.vector.tensor_add(xsT[0:D, :], xsT[0:D, :], dT[:])

            # r0 = rowsum of xs0 (before any delta is applied)
            r0 = miscp.tile([D, 1], F32, tag="r0")
            nc.vector.tensor_reduce(r0[:], xsT[0:D, :],
                                    axis=mybir.AxisListType.X,
                                    op=ALU.add)

            rds = miscp.tile([D, 3], F32, tag="rds")  # per-layer dl rowsums

            # ---------------- GNN L1 ----------------
            dl1 = gnn_layer(0)
            stage_delta(dl1, 0)
            nc.gpsimd.collective_compute(
                "AllGather", ALU.bypass,
                ins=[cc_in[0].opt()], outs=[cc_out[0].opt()],
                replica_groups=rg)
            nc.vector.tensor_reduce(rds[:, 0:1], dl1[:],
                                    axis=mybir.AxisListType.X, op=ALU.add)

            # conv L1 while AG1 is in flight
            build_x12(1)
            conv_layer(1)

            apply_delta(0)

            # ---------------- GNN L2 ----------------
            dl2 = gnn_layer(1)
            stage_delta(dl2, 1)
            nc.gpsimd.collective_compute(
                "AllGather", ALU.bypass,
                ins=[cc_in[1].opt()], outs=[cc_out[1].opt()],
                replica_groups=rg)
            nc.vector.tensor_reduce(rds[:, 1:2], dl2[:],
                                    axis=mybir.AxisListType.X, op=ALU.add)

            # conv L2 while AG2 is in flight
            build_x12(2)
            conv_layer(2)

            apply_delta(1)

            # ---------------- GNN L3 + compound ----------------
            dl3 = gnn_layer(2)
            nc.vector.tensor_reduce(rds[:, 2:3], dl3[:],
                                    axis=mybir.AxisListType.X, op=ALU.add)
            # part_c = (r0/8 + sum_l rowsum(dl_l)) / NA
            part_c = miscp.tile([D, 8], F32, tag="pc")
            nc.vector.memset(part_c[:], 0.0)
            racc = miscp.tile([D, 1], F32, tag="racc")
            nc.vector.tensor_add(racc[:], rds[:, 0:1], rds[:, 1:2])
            nc.vector.tensor_add(racc[:], racc[:], rds[:, 2:3])
            nc.vector.scalar_tensor_tensor(
                racc[:], r0[:], 1.0 / NCORES, racc[:],
                op0=ALU.mult, op1=ALU.add)
            nc.vector.tensor_scalar_mul(part_c[:, 0:1], racc[:], 1.0 / NA)
            nc.gpsimd.dma_start(ar_c_in[:], part_c[:])
            nc.gpsimd.collective_compute(
                "AllReduce", ALU.add,
                ins=[ar_c_in.opt()], outs=[ar_c_out.opt()],
                replica_groups=rg)

            # conv L3 while the compound AllReduce is in flight
            build_x12(3)
            conv_layer(3)

            # -------- attention pass 1: hsp for all tiles (no comp dep) ----
            hspA = attp.tile([D, NT * T], BF, tag="hspA")
            for t in range(NT):
                b0 = HALO + t * T
                ps1 = ps_mix.tile([D, T], F32, tag="mix", name="ps1")
                nc.tensor.matmul(ps1[:], watT_bf, ximg[:, b0:b0 + T])
                dst = hspA[:, t * T:(t + 1) * T]
                if VRELU and t % 2 == 1:
                    nc.vector.tensor_scalar(dst, ps1[:], batt, 0.0,
                                            op0=ALU.add, op1=ALU.max)
                else:
                    nc.scalar.activation(dst, ps1[:], AF.Relu, bias=batt)

            # -------- compound + h --------
            comp = miscp.tile([D, 1], F32, tag="comp")
            nc.gpsimd.dma_start(comp[:], ar_c_out[:, 0:1])
            h_ps = ps_sm.tile([20, 1], F32, tag="tiny")
            nc.tensor.matmul(h_ps[0:D, :], watT, comp[:])
            h_sb = miscp.tile([D, 1], BF, tag="hsb")
            nc.scalar.activation(h_sb[:], h_ps[0:D, :], AF.Relu, bias=batt)

            # -------- attention pass 2 --------
            wrall = attp.tile([1, NT * T], BF, tag="wrall")
            pp = miscp.tile([D, NT], F32, tag="pp")
            ys = [attp.tile([D, T], BF, tag=f"ys{i}", name=f"ys{i}")
                  for i in range(2)]
            for t in range(NT):
                wrps = ps_wr.tile([1, T], F32, tag="wr")
                nc.tensor.matmul(wrps[:], h_sb[:],
                                 hspA[:, t * T:(t + 1) * T])
                wr = wrall[0:1, t * T:(t + 1) * T]
                nc.scalar.activation(wr, wrps[:], AF.Tanh)
                ps3 = ps_mix.tile([D, T], F32, tag="mix", name="ps3")
                nc.tensor.matmul(ps3[:], ones_bf, wr)
                yt = ys[t % 2]
                if TTR:
                    nc.vector.tensor_tensor_reduce(
                        yt[:], hspA[:, t * T:(t + 1) * T], ps3[:], 1.0, 0.0,
                        op0=ALU.mult, op1=ALU.add, accum_out=pp[:, t:t + 1])
                else:
                    nc.vector.tensor_mul(yt[:], hspA[:, t * T:(t + 1) * T],
                                         ps3[:])
                    nc.vector.tensor_reduce(pp[:, t:t + 1], yt[:],
                                            axis=mybir.AxisListType.X,
                                            op=ALU.add)

            part_p = miscp.tile([D, 8], F32, tag="ppad")
            nc.vector.memset(part_p[:], 0.0)
            nc.vector.tensor_reduce(part_p[:, 0:1], pp[:],
                                    axis=mybir.AxisListType.X,
                                    op=ALU.add)
            nc.gpsimd.dma_start(ar_p_in[:], part_p[:])
            nc.gpsimd.collective_compute(
                "AllReduce", ALU.add,
                ins=[ar_p_in.opt()], outs=[ar_p_out.opt()],
                replica_groups=rg)
            prot = miscp.tile([D, 1], F32, tag="prot")
            nc.gpsimd.dma_start(prot[:], ar_p_out[:, 0:1])

            # ---------------- fusion MLP ----------------
            woa0 = sm[0:D, 11:31]
            wob0 = sm[0:D, 31:51]
            bo0 = sm[0:20, 51:52]
            woT1 = sm[0:20, 52:72]
            woT2 = sm[0:20, 72:92]
            bo1 = sm[0:20, 92:93]
            bo2 = sm[0:20, 93:94]
            wiT = sm[0:20, 94:96]
            bi = sm[0:2, 96:97]

            f_ps = ps_sm.tile([20, 1], F32, tag="tiny")
            nc.tensor.matmul(f_ps[:], woa0, comp[:], start=True, stop=False)
            nc.tensor.matmul(f_ps[:], wob0, prot[:], start=False, stop=True)
            cat1 = miscp.tile([20, 1], F32, tag="cat1")
            nc.scalar.activation(cat1[:], f_ps[:], AF.Relu, bias=bo0)
            f_ps2 = ps_sm.tile([20, 1], F32, tag="tiny")
            nc.tensor.matmul(f_ps2[:], woT1, cat1[:])
            cat2 = miscp.tile([20, 1], F32, tag="cat2")
            nc.scalar.activation(cat2[:], f_ps2[:], AF.Relu, bias=bo1)
            f_ps3 = ps_sm.tile([20, 1], F32, tag="tiny")
            nc.tensor.matmul(f_ps3[:], woT2, cat2[:])
            cat3 = miscp.tile([20, 1], F32, tag="cat3")
            nc.scalar.activation(cat3[:], f_ps3[:], AF.Relu, bias=bo2)
            o_ps = ps_sm.tile([20, 1], F32, tag="tiny")
            nc.tensor.matmul(o_ps[0:2, :], wiT, cat3[:])
            o_sb = miscp.tile([2, 1], F32, tag="osb")
            nc.scalar.activation(o_sb[:], o_ps[0:2, :], AF.Identity, bias=bi)
            nc.sync.dma_start(out_d[:], o_sb[:])

    nc.compile()
    _BUILD_CACHE[key] = nc
    return nc


def _host_prep(fingerprints, adjacency, words, embed_fp, embed_word,
               W_gnn_w, W_gnn_b, W_cnn_w, W_cnn_b, W_att_w, W_att_b,
               W_out_w, W_out_b, W_int_w, W_int_b):
    f32 = np.float32
    fingerprints = np.asarray(fingerprints).astype(np.int64)
    words = np.asarray(words).astype(np.int64)
    adjacency = np.asarray(adjacency, dtype=f32)
    embed_fp = np.asarray(embed_fp, dtype=f32)
    embed_word = np.asarray(embed_word, dtype=f32)
    W_gnn_w = np.asarray(W_gnn_w, dtype=f32)
    W_gnn_b = np.asarray(W_gnn_b, dtype=f32)
    W_cnn_w = np.asarray(W_cnn_w, dtype=f32)
    W_cnn_b = np.asarray(W_cnn_b, dtype=f32)
    W_att_w = np.asarray(W_att_w, dtype=f32)
    W_att_b = np.asarray(W_att_b, dtype=f32)
    W_out_w = np.asarray(W_out_w, dtype=f32)
    W_out_b = np.asarray(W_out_b, dtype=f32)
    W_int_w = np.asarray(W_int_w, dtype=f32)
    W_int_b = np.asarray(W_int_b, dtype=f32)

    # xsT0 [11, NA]: gathered compound embeddings, transposed + ones row
    xs0 = embed_fp[fingerprints]                       # [NA, D]
    xsT0 = np.zeros((11, NA), dtype=f32)
    xsT0[0:D] = xs0.T
    xsT0[D] = 1.0
    xsT0 = xsT0.astype(BF16)

    # adjacency row-shards, transposed, pre-chunked for SBUF, bf16:
    # a_p[p, c*T + j] = A[core*R + j, c*128 + p]
    a_p = []
    for c in range(NCORES):
        at = np.ascontiguousarray(adjacency[c * R:(c + 1) * R, :].T)  # [NA,R]
        ap = at.reshape(NCH, 128, R).transpose(1, 0, 2).reshape(128, NCH * R)
        a_p.append(ap.astype(BF16))

    # protein image shards with halo, transposed, bf16
    ws = embed_word[words]                             # [L, D]
    wspad = np.zeros((L + 2 * HALO, D), dtype=f32)
    wspad[HALO:HALO + L] = ws
    wsT = [np.ascontiguousarray(wspad[c * LC:c * LC + LBUF].T).astype(BF16)
           for c in range(NCORES)]

    # host-prestacked x12 for conv layer 1
    x12h = []
    for c in range(NCORES):
        xi = np.zeros((120, LBUF), dtype=f32)
        wt = wsT[c].astype(f32)
        for pp in range(12):
            xi[10 * pp:10 * pp + 10, 0:LBUF - pp] = wt[:, pp:LBUF]
        x12h.append(xi.astype(BF16))

    # conv Toeplitz groups
    gmv = np.zeros((120, GM_COLS), dtype=f32)
    for l in range(3):
        ker = W_cnn_w[l, 0, 0]                         # [23, 23]
        g0 = np.zeros((120, D), dtype=f32)
        g1 = np.zeros((110, D), dtype=f32)
        for w in range(D):
            for j in range(D):
                kx = w - j + PAD
                for p in range(12):
                    g0[10 * p + w, j] = ker[p, kx]
                for p in range(11):
                    g1[10 * p + w, j] = ker[p + 12, kx]
        gmv[:, 20 * l:20 * l + 10] = g0
        gmv[0:110, 20 * l + 10:20 * l + 20] = g1
    gmv[0:D, 60:70] = W_att_w.T
    for l in range(3):
        gmv[0:D, 70 + 10 * l:80 + 10 * l] = W_gnn_w[l].T
        gmv[D, 70 + 10 * l:80 + 10 * l] = W_gnn_b[l]
    gmv[0:1, 100:110] = 1.0 / L
    gmv = gmv.astype(BF16)

    sm = np.zeros((128, SM_COLS), dtype=f32)
    sm[0:D, 0:10] = W_att_w.T
    sm[0:D, 10] = W_att_b
    sm[0:D, 11:31] = W_out_w[0][:, 0:D].T
    sm[0:D, 31:51] = W_out_w[0][:, D:2 * D].T
    sm[0:20, 51] = W_out_b[0]
    sm[0:20, 52:72] = W_out_w[1].T
    sm[0:20, 72:92] = W_out_w[2].T
    sm[0:20, 92] = W_out_b[1]
    sm[0:20, 93] = W_out_b[2]
    sm[0:20, 94:96] = W_int_w.T
    sm[0:2, 96] = W_int_b
    for i in range(3):
        sm[0:D, 97 + i] = W_cnn_b[i]

    in_maps = []
    for c in range(NCORES):
        in_maps.append({
            "xsT0": xsT0,
            "a_p": a_p[c],
            "wsT": wsT[c],
            "x12h": x12h[c],
            "gm": gmv,
            "smalls": sm,
        })
    return in_maps


def kernel(**inputs):
    in_maps = _host_prep(**inputs)
    nc = build_program()
    res = run_bass_kernel_spmd(nc, in_maps, list(range(NCORES)))
    return np.asarray(res.results[0]["out"], dtype=np.float32)


# revision 17
# speedup vs baseline: 1.2054x; 1.2054x over previous
"""Trainium2 Bass kernel for nn_CPI_CLS_49478023250092 (gnn_message_passing).

Strategy (8 cores, SPMD):
  - GNN: adjacency row-sharded; each core holds A_c.T (4096x512, bf16,
    pre-chunked on host into [128, 32*512]) resident in SBUF, computes
    delta.T = (A_c @ hs).T = sum_k hs_chunk.T @ A_cT_chunk on the tensor
    engine in bf16; per-layer AllGather of the bf16 [10,512] delta
    recovers the full xs.T on every core.  3 layers.
  - Protein conv: L-sharded with 33-col halos (zero at global edges).
    23x23 conv over a [L,10] image = TWO accumulating bf16 matmuls per
    512-col tile against a 12-shift stacked image X12 [120, L].
  - Attention: two-pass tail.  Pass 1 (overlaps the compound AllReduce):
    hsp = relu(W_att @ conv_out + b) for all 16 tiles.  Pass 2: 16
    weight matmuls into one [16,512] PSUM bank, one batched tanh, then
    per tile a broadcast matmul + fused multiply-reduce
    (tensor_tensor_reduce) accumulating the weighted mean.
  - Fusion MLP in f32 on every core; tiny AllReduces for compound and
    protein partial means.
  - Host side does only data movement: embedding gathers, sharding,
    transposition, Toeplitz construction, dtype casts.
"""

import sys
import os

for _p in ("/opt/trn_rl_repo",):
    if _p not in sys.path and os.path.isdir(_p):
        sys.path.insert(0, _p)

import numpy as np
import ml_dtypes

import concourse.bacc as bacc
import concourse.mybir as mybir
from concourse import tile
from concourse.bass_utils import run_bass_kernel_spmd

BF16 = ml_dtypes.bfloat16

NCORES = 8
NA = 4096          # atoms
D = 10             # embed dim
L = 65536          # words
KK = 23            # conv kernel
PAD = 11
R = NA // NCORES   # 512 adjacency rows per core
NCH = NA // 128    # 32 k-chunks
LC = L // NCORES   # 8192 conv columns per core
HALO = 33
LBUF = LC + 2 * HALO   # 8258
T = 512            # free-dim tile
NT = LC // T       # 16 attention tiles

F32 = mybir.dt.float32
BF = mybir.dt.bfloat16

# ---- smalls layout (f32 [128, 100]) ----
# cols 0-9   : watT f32 [10,10]
# col  10    : batt [10,1]
# cols 11-30 : woa0 [10,20] = W_out0[:, :10].T
# cols 31-50 : wob0 [10,20] = W_out0[:, 10:].T
# col  51    : bo0 [20,1]
# cols 52-71 : woT1 [20,20]
# cols 72-91 : woT2 [20,20]
# col 92     : bo1 ; col 93 : bo2
# cols 94-95 : wiT [20,2]
# col  96    : bi [2,1]
# cols 97-99 : conv biases [10,1] for layers 1..3
SM_COLS = 100
# ---- gm layout (bf16 [120, 110]) ----
# cols 20l+0..9  : G0_l [120,10] ; cols 20l+10..19 : G1_l [110,10] (padded)
# cols 60-69     : watT bf16 [10,10]
# cols 70+10l    : wgT_bf[l] [11,10] (W_gnn_w[l].T stacked with bias row)
# cols 100-109   : ones_sc [1,10] at partition 0 (value 1/65536)
GM_COLS = 110

_BUILD_CACHE = {}


def _conv_spans():
    """Per conv layer (1..3): (in_lo, in_hi, out_lo, out_hi) in buffer coords."""
    spans = []
    for l in (1, 2, 3):
        in_lo = 11 * (l - 1)
        in_hi = LBUF - 11 * (l - 1)
        out_lo = 11 * l
        out_hi = LBUF - 11 * l
        spans.append((in_lo, in_hi, out_lo, out_hi))
    return spans


def _tiles(lo, hi, step):
    out = []
    c = lo
    while c < hi:
        out.append((c, min(step, hi - c)))
        c += step
    return out


def build_program():
    # tensor_tensor_reduce hangs real HW (works in CoreSim) — keep it off
    TTR = os.environ.get("K_TTR", "0") == "1"
    VRELU = os.environ.get("K_VRELU", "1") == "1"  # DVE relu variants
    key = ("nc", TTR, VRELU)
    if key in _BUILD_CACHE:
        return _BUILD_CACHE[key]

    nc = bacc.Bacc("TRN2", target_bir_lowering=False, debug=False,
                   num_devices=NCORES)

    xsT0 = nc.dram_tensor("xsT0", [11, NA], BF, kind="ExternalInput").ap()
    a_p = nc.dram_tensor("a_p", [128, NCH * T], BF, kind="ExternalInput").ap()
    wsT = nc.dram_tensor("wsT", [D, LBUF], BF, kind="ExternalInput").ap()
    x12h = nc.dram_tensor("x12h", [120, LBUF], BF, kind="ExternalInput").ap()
    gm = nc.dram_tensor("gm", [120, GM_COLS], BF, kind="ExternalInput").ap()
    smalls = nc.dram_tensor("smalls", [128, SM_COLS], F32,
                            kind="ExternalInput").ap()
    out_d = nc.dram_tensor("out", [1, 2], F32, kind="ExternalOutput").ap()

    spans = _conv_spans()
    rg = [list(range(NCORES))]
    AF = mybir.ActivationFunctionType
    ALU = mybir.AluOpType

    with tile.TileContext(nc) as tc:
        with (
            tc.tile_pool(name="const", bufs=1) as constp,
            tc.tile_pool(name="abuf", bufs=1) as abufp,
            tc.tile_pool(name="ximg", bufs=1) as ximgp,
            tc.tile_pool(name="x12", bufs=1) as x12p,
            tc.tile_pool(name="hs", bufs=1) as hsp_pool,
            tc.tile_pool(name="dl", bufs=2) as dlp,
            tc.tile_pool(name="att", bufs=2) as attp,
            tc.tile_pool(name="misc", bufs=2) as miscp,
            tc.tile_pool(name="ps_mix", bufs=4, space="PSUM") as ps_mix,
            tc.tile_pool(name="ps_dl", bufs=1, space="PSUM") as ps_dl,
            tc.tile_pool(name="ps_sm", bufs=1, space="PSUM") as ps_sm,
            tc.tile_pool(name="ps_wr", bufs=2, space="PSUM") as ps_wr,
            tc.tile_pool(name="dram", bufs=1, space="DRAM") as dram,
        ):
            # ---------------- warmup collective (first!) ----------------
            # the first collective pays a large fixed init + skew barrier;
            # trigger it before anything else so it overlaps the loads
            warm = miscp.tile([1, 8], F32, tag="warm")
            nc.vector.memset(warm[:], 0.0)
            wr_in = dram.tile([1, 8], F32, tag="wrin")
            wr_out = dram.tile([1, 8], F32, tag="wrout")
            nc.scalar.dma_start(wr_in[:], warm[:])
            nc.gpsimd.collective_compute(
                "AllReduce", ALU.add,
                ins=[wr_in.opt()], outs=[wr_out.opt()],
                replica_groups=rg)

            # ---------------- load phase ----------------
            sm = constp.tile([128, SM_COLS], F32, tag="sm")
            nc.sync.dma_start(sm[:], smalls[:])
            gmt = constp.tile([120, GM_COLS], BF, tag="gm")
            nc.sync.dma_start(gmt[:], gm[:])
            xsT = constp.tile([11, NA], BF, tag="xsT")
            nc.sync.dma_start(xsT[:], xsT0[:])
            ximg = ximgp.tile([D, LBUF], BF, tag="ximg")
            nc.sync.dma_start(ximg[:], wsT[:])

            a_sb = abufp.tile([128, NCH * T], BF, tag="a")
            ADMA = 4  # chunks per DMA
            for i, c in enumerate(range(0, NCH, ADMA)):
                eng = nc.sync if i % 2 == 0 else nc.scalar
                eng.dma_start(a_sb[:, c * T:(c + ADMA) * T],
                              a_p[:, c * T:(c + ADMA) * T])

            x12 = x12p.tile([120, LBUF], BF, tag="x12")

            # collective bounce buffers
            cc_in = [dram.tile([D, T], BF, tag=f"ccin{i}",
                               name=f"ccin{i}") for i in range(2)]
            cc_out = [dram.tile([8 * D, T], BF, tag=f"ccout{i}",
                                name=f"ccout{i}") for i in range(2)]
            ar_c_in = dram.tile([D, 8], F32, tag="arcin")
            ar_c_out = dram.tile([D, 8], F32, tag="arcout")
            ar_p_in = dram.tile([D, 8], F32, tag="arpin")
            ar_p_out = dram.tile([D, 8], F32, tag="arpout")

            wgT = [gmt[0:11, 70 + 10 * l:80 + 10 * l] for l in range(3)]
            watT = sm[0:D, 0:10]
            batt = sm[0:D, 10:11]
            watT_bf = gmt[0:D, 60:70]
            ones_bf = gmt[0:1, 100:110]
            cbias = [sm[0:D, 97 + i:98 + i] for i in range(3)]

            def build_x12(l):
                if l == 1:
                    # layer-1 stack comes pre-built from the host: one big
                    # DMA with 128 descriptors spreads across all engines
                    nc.sync.dma_start(x12[:], x12h[:])
                    return
                # on-device rebuild on the two HWDGE queues
                in_lo, in_hi, _, _ = spans[l - 1]
                engs = [nc.sync, nc.scalar]
                for p in range(12):
                    engs[p % 2].dma_start(
                        x12[10 * p:10 * p + 10, in_lo:in_hi - p],
                        ximg[:, in_lo + p:in_hi])

            _cv_count = [0]

            def conv_layer(l):
                in_lo, in_hi, out_lo, out_hi = spans[l - 1]
                g0 = gmt[0:120, 20 * (l - 1):20 * (l - 1) + 10]
                g1 = gmt[0:110, 20 * (l - 1) + 10:20 * (l - 1) + 20]
                for (b0, tw) in _tiles(out_lo, out_hi, T):
                    ps = ps_mix.tile([D, T], F32, tag="mix")
                    nc.tensor.matmul(ps[:, :tw], g0,
                                     x12[0:120, b0 - 11:b0 - 11 + tw],
                                     start=True, stop=False)
                    nc.tensor.matmul(ps[:, :tw], g1,
                                     x12[0:110, b0 + 1:b0 + 1 + tw],
                                     start=False, stop=True)
                    _cv_count[0] += 1
                    if VRELU and _cv_count[0] % 2 == 1:
                        nc.vector.tensor_scalar(ximg[:, b0:b0 + tw],
                                                ps[:, :tw], cbias[l - 1], 0.0,
                                                op0=ALU.add, op1=ALU.max)
                    else:
                        nc.scalar.activation(ximg[:, b0:b0 + tw], ps[:, :tw],
                                             AF.Relu, bias=cbias[l - 1])

            def gnn_layer(l):
                """Interleaved hs+delta matmuls; returns delta psum."""
                hs_sb = hsp_pool.tile([128, NCH * D], BF, tag="hs")
                dl_ps = ps_dl.tile([D, T], F32, tag="dl")
                for c in range(NCH):
                    hp = ps_mix.tile([128, D], F32, tag="mix", name="hp")
                    nc.tensor.matmul(hp[:], xsT[:, 128 * c:128 * (c + 1)],
                                     wgT[l])
                    dst = hs_sb[:, D * c:D * (c + 1)]
                    if VRELU:
                        nc.vector.tensor_scalar_max(dst, hp[:], 0.0)
                    else:
                        nc.scalar.activation(dst, hp[:], AF.Relu)
                    nc.tensor.matmul(dl_ps[:], dst,
                                     a_sb[:, T * c:T * (c + 1)],
                                     start=(c == 0), stop=(c == NCH - 1))
                return dl_ps

            def stage_delta(dl_ps, idx):
                dcp = dlp.tile([D, T], BF, tag="dcp")
                nc.vector.tensor_scalar_add(dcp[:], dl_ps[:], 0.0)
                nc.gpsimd.dma_start(cc_in[idx][:], dcp[:])

            def apply_delta(idx):
                """DMA gathered deltas back and add into xsT."""
                dT = dlp.tile([D, NA], BF, tag="dT")
                nc.sync.dma_start(
                    dT[:].rearrange("j (r n) -> j r n", r=NCORES),
                    cc_out[idx][:].rearrange("(r j) n -> j r n", j=D))
                nc.vector.tensor_add(xsT[0:D, :], xsT[0:D, :], dT[:])

            # r0 = rowsum of xs0 (before any delta is applied)
            r0 = miscp.tile([D, 1], F32, tag="r0")
            nc.vector.tensor_reduce(r0[:], xsT[0:D, :],
                                    axis=mybir.AxisListType.X,
                                    op=ALU.add)

            rds = miscp.tile([D, 3], F32, tag="rds")  # per-layer dl rowsums

            # ---------------- GNN L1 ----------------
            dl1 = gnn_layer(0)
            stage_delta(dl1, 0)
            nc.gpsimd.collective_compute(
                "AllGather", ALU.bypass,
                ins=[cc_in[0].opt()], outs=[cc_out[0].opt()],
                replica_groups=rg)
            nc.vector.tensor_reduce(rds[:, 0:1], dl1[:],
                                    axis=mybir.AxisListType.X, op=ALU.add)

            # conv L1 while AG1 is in flight
            build_x12(1)
            conv_layer(1)

            apply_delta(0)

            # ---------------- GNN L2 ----------------
            dl2 = gnn_layer(1)
            stage_delta(dl2, 1)
            nc.gpsimd.collective_compute(
                "AllGather", ALU.bypass,
                ins=[cc_in[1].opt()], outs=[cc_out[1].opt()],
                replica_groups=rg)
            nc.vector.tensor_reduce(rds[:, 1:2], dl2[:],
                                    axis=mybir.AxisListType.X, op=ALU.add)

            # conv L2 while AG2 is in flight
            build_x12(2)
            conv_layer(2)

            apply_delta(1)

            # ---------------- GNN L3 + compound ----------------
            dl3 = gnn_layer(2)
            nc.vector.tensor_reduce(rds[:, 2:3], dl3[:],
                                    axis=mybir.AxisListType.X, op=ALU.add)
            # part_c = (r0/8 + sum_l rowsum(dl_l)) / NA
            part_c = miscp.tile([D, 8], F32, tag="pc")
            nc.vector.memset(part_c[:], 0.0)
            racc = miscp.tile([D, 1], F32, tag="racc")
            nc.vector.tensor_add(racc[:], rds[:, 0:1], rds[:, 1:2])
            nc.vector.tensor_add(racc[:], racc[:], rds[:, 2:3])
            nc.vector.scalar_tensor_tensor(
                racc[:], r0[:], 1.0 / NCORES, racc[:],
                op0=ALU.mult, op1=ALU.add)
            nc.vector.tensor_scalar_mul(part_c[:, 0:1], racc[:], 1.0 / NA)
            nc.gpsimd.dma_start(ar_c_in[:], part_c[:])
            nc.gpsimd.collective_compute(
                "AllReduce", ALU.add,
                ins=[ar_c_in.opt()], outs=[ar_c_out.opt()],
                replica_groups=rg)

            # conv L3 while the compound AllReduce is in flight
            build_x12(3)
            conv_layer(3)

            # -------- attention pass 1: hsp for all tiles (no comp dep) ----
            hspA = attp.tile([D, NT * T], BF, tag="hspA")
            for t in range(NT):
                b0 = HALO + t * T
                ps1 = ps_mix.tile([D, T], F32, tag="mix", name="ps1")
                nc.tensor.matmul(ps1[:], watT_bf, ximg[:, b0:b0 + T])
                dst = hspA[:, t * T:(t + 1) * T]
                if VRELU and t % 2 == 1:
                    nc.vector.tensor_scalar(dst, ps1[:], batt, 0.0,
                                            op0=ALU.add, op1=ALU.max)
                else:
                    nc.scalar.activation(dst, ps1[:], AF.Relu, bias=batt)

            # -------- compound + h --------
            comp = miscp.tile([D, 1], F32, tag="comp")
            nc.gpsimd.dma_start(comp[:], ar_c_out[:, 0:1])
            h_ps = ps_sm.tile([20, 1], F32, tag="tiny")
            nc.tensor.matmul(h_ps[0:D, :], watT, comp[:])
            h_sb = miscp.tile([D, 1], BF, tag="hsb")
            nc.scalar.activation(h_sb[:], h_ps[0:D, :], AF.Relu, bias=batt)

            # -------- attention pass 2 --------
            wrall = attp.tile([1, NT * T], BF, tag="wrall")
            pp = miscp.tile([D, NT], F32, tag="pp")
            ys = [attp.tile([D, T], BF, tag=f"ys{i}", name=f"ys{i}")
                  for i in range(2)]
            for t in range(NT):
                wrps = ps_wr.tile([1, T], F32, tag="wr")
                nc.tensor.matmul(wrps[:], h_sb[:],
                                 hspA[:, t * T:(t + 1) * T])
                wr = wrall[0:1, t * T:(t + 1) * T]
                nc.scalar.activation(wr, wrps[:], AF.Tanh)
                ps3 = ps_mix.tile([D, T], F32, tag="mix", name="ps3")
                nc.tensor.matmul(ps3[:], ones_bf, wr)
                yt = ys[t % 2]
                if TTR:
                    nc.vector.tensor_tensor_reduce(
                        yt[:], hspA[:, t * T:(t + 1) * T], ps3[:], 1.0, 0.0,
                        op0=ALU.mult, op1=ALU.add, accum_out=pp[:, t:t + 1])
                else:
                    nc.vector.tensor_mul(yt[:], hspA[:, t * T:(t + 1) * T],
                                         ps3[:])
                    nc.vector.tensor_reduce(pp[:, t:t + 1], yt[:],
                                            axis=mybir.AxisListType.X,
                                            op=ALU.add)

            part_p = miscp.tile([D, 8], F32, tag="ppad")
            nc.vector.memset(part_p[:], 0.0)
            nc.vector.tensor_reduce(part_p[:, 0:1], pp[:],
                                    axis=mybir.AxisListType.X,
                                    op=ALU.add)
            nc.gpsimd.dma_start(ar_p_in[:], part_p[:])
            nc.gpsimd.collective_compute(
                "AllReduce", ALU.add,
                ins=[ar_p_in.opt()], outs=[ar_p_out.opt()],
                replica_groups=rg)
            prot = miscp.tile([D, 1], F32, tag="prot")
            nc.gpsimd.dma_start(prot[:], ar_p_out[:, 0:1])

            # ---------------- fusion MLP ----------------
            woa0 = sm[0:D, 11:31]
            wob0 = sm[0:D, 31:51]
            bo0 = sm[0:20, 51:52]
            woT1 = sm[0:20, 52:72]
            woT2 = sm[0:20, 72:92]
            bo1 = sm[0:20, 92:93]
            bo2 = sm[0:20, 93:94]
            wiT = sm[0:20, 94:96]
            bi = sm[0:2, 96:97]

            f_ps = ps_sm.tile([20, 1], F32, tag="tiny")
            nc.tensor.matmul(f_ps[:], woa0, comp[:], start=True, stop=False)
            nc.tensor.matmul(f_ps[:], wob0, prot[:], start=False, stop=True)
            cat1 = miscp.tile([20, 1], F32, tag="cat1")
            nc.scalar.activation(cat1[:], f_ps[:], AF.Relu, bias=bo0)
            f_ps2 = ps_sm.tile([20, 1], F32, tag="tiny")
            nc.tensor.matmul(f_ps2[:], woT1, cat1[:])
            cat2 = miscp.tile([20, 1], F32, tag="cat2")
            nc.scalar.activation(cat2[:], f_ps2[:], AF.Relu, bias=bo1)
            f_ps3 = ps_sm.tile([20, 1], F32, tag="tiny")
            nc.tensor.matmul(f_ps3[:], woT2, cat2[:])
            cat3 = miscp.tile([20, 1], F32, tag="cat3")
            nc.scalar.activation(cat3[:], f_ps3[:], AF.Relu, bias=bo2)
            o_ps = ps_sm.tile([20, 1], F32, tag="tiny")
            nc.tensor.matmul(o_ps[0:2, :], wiT, cat3[:])
            o_sb = miscp.tile([2, 1], F32, tag="osb")
            nc.scalar.activation(o_sb[:], o_ps[0:2, :], AF.Identity, bias=bi)
            nc.sync.dma_start(out_d[:], o_sb[:])

    nc.compile()
    _BUILD_CACHE[key] = nc
    return nc


def _host_prep(fingerprints, adjacency, words, embed_fp, embed_word,
               W_gnn_w, W_gnn_b, W_cnn_w, W_cnn_b, W_att_w, W_att_b,
               W_out_w, W_out_b, W_int_w, W_int_b):
    f32 = np.float32
    fingerprints = np.asarray(fingerprints).astype(np.int64)
    words = np.asarray(words).astype(np.int64)
    adjacency = np.asarray(adjacency, dtype=f32)
    embed_fp = np.asarray(embed_fp, dtype=f32)
    embed_word = np.asarray(embed_word, dtype=f32)
    W_gnn_w = np.asarray(W_gnn_w, dtype=f32)
    W_gnn_b = np.asarray(W_gnn_b, dtype=f32)
    W_cnn_w = np.asarray(W_cnn_w, dtype=f32)
    W_cnn_b = np.asarray(W_cnn_b, dtype=f32)
    W_att_w = np.asarray(W_att_w, dtype=f32)
    W_att_b = np.asarray(W_att_b, dtype=f32)
    W_out_w = np.asarray(W_out_w, dtype=f32)
    W_out_b = np.asarray(W_out_b, dtype=f32)
    W_int_w = np.asarray(W_int_w, dtype=f32)
    W_int_b = np.asarray(W_int_b, dtype=f32)

    # xsT0 [11, NA]: gathered compound embeddings, transposed + ones row
    xs0 = embed_fp[fingerprints]                       # [NA, D]
    xsT0 = np.zeros((11, NA), dtype=f32)
    xsT0[0:D] = xs0.T
    xsT0[D] = 1.0
    xsT0 = xsT0.astype(BF16)

    # adjacency row-shards, transposed, pre-chunked for SBUF, bf16:
    # a_p[p, c*T + j] = A[core*R + j, c*128 + p]
    a_p = []
    for c in range(NCORES):
        at = np.ascontiguousarray(adjacency[c * R:(c + 1) * R, :].T)  # [NA,R]
        ap = at.reshape(NCH, 128, R).transpose(1, 0, 2).reshape(128, NCH * R)
        a_p.append(ap.astype(BF16))

    # protein image shards with halo, transposed, bf16
    ws = embed_word[words]                             # [L, D]
    wspad = np.zeros((L + 2 * HALO, D), dtype=f32)
    wspad[HALO:HALO + L] = ws
    wsT = [np.ascontiguousarray(wspad[c * LC:c * LC + LBUF].T).astype(BF16)
           for c in range(NCORES)]

    # host-prestacked x12 for conv layer 1
    x12h = []
    for c in range(NCORES):
        xi = np.zeros((120, LBUF), dtype=f32)
        wt = wsT[c].astype(f32)
        for pp in range(12):
            xi[10 * pp:10 * pp + 10, 0:LBUF - pp] = wt[:, pp:LBUF]
        x12h.append(xi.astype(BF16))

    # conv Toeplitz groups
    gmv = np.zeros((120, GM_COLS), dtype=f32)
    for l in range(3):
        ker = W_cnn_w[l, 0, 0]                         # [23, 23]
        g0 = np.zeros((120, D), dtype=f32)
        g1 = np.zeros((110, D), dtype=f32)
        for w in range(D):
            for j in range(D):
                kx = w - j + PAD
                for p in range(12):
                    g0[10 * p + w, j] = ker[p, kx]
                for p in range(11):
                    g1[10 * p + w, j] = ker[p + 12, kx]
        gmv[:, 20 * l:20 * l + 10] = g0
        gmv[0:110, 20 * l + 10:20 * l + 20] = g1
    gmv[0:D, 60:70] = W_att_w.T
    for l in range(3):
        gmv[0:D, 70 + 10 * l:80 + 10 * l] = W_gnn_w[l].T
        gmv[D, 70 + 10 * l:80 + 10 * l] = W_gnn_b[l]
    gmv[0:1, 100:110] = 1.0 / L
    gmv = gmv.astype(BF16)

    sm = np.zeros((128, SM_COLS), dtype=f32)
    sm[0:D, 0:10] = W_att_w.T
    sm[0:D, 10] = W_att_b
    sm[0:D, 11:31] = W_out_w[0][:, 0:D].T
    sm[0:D, 31:51] = W_out_w[0][:, D:2 * D].T
    sm[0:20, 51] = W_out_b[0]
    sm[0:20, 52:72] = W_out_w[1].T
    sm[0:20, 72:92] = W_out_w[2].T
    sm[0:20, 92] = W_out_b[1]
    sm[0:20, 93] = W_out_b[2]
    sm[0:20, 94:96] = W_int_w.T
    sm[0:2, 96] = W_int_b
    for i in range(3):
        sm[0:D, 97 + i] = W_cnn_b[i]

    in_maps = []
    for c in range(NCORES):
        in_maps.append({
            "xsT0": xsT0,
            "a_p": a_p[c],
            "wsT": wsT[c],
            "x12h": x12h[c],
            "gm": gmv,
            "smalls": sm,
        })
    return in_maps


def kernel(**inputs):
    in_maps = _host_prep(**inputs)
    nc = build_program()
    res = run_bass_kernel_spmd(nc, in_maps, list(range(NCORES)))
    return np.asarray(res.results[0]["out"], dtype=np.float32)
